# revision 1
# baseline (speedup 1.0000x reference)
"""Trainium2 Bass kernel for nn_CrossAttention_86165633892747.

Math: seq_len_q = seq_len_kv = 1, so softmax over the length-1 key axis is
exactly 1.0 and attn_out == v.  The whole module collapses to

    out = (chem_16 @ Wv.T + bv) @ Wout.T + bout
        = chem_16 @ (Wout @ Wv).T + (Wout @ bv + bout)

i.e. a single per-row 16x16 linear map.  fp_16 / Wq / Wk / bq / bk are dead.

Device strategy (pure data parallel over 8 cores, B/8 = 262144 rows each):
  - View the row-major (R,16) shard as flat 128x128 fp32 tiles where each
    SBUF partition holds 128 consecutive floats = 8 consecutive rows.
  - PE transpose the tile (fp32 exact, identity matmul) -> X^T in PSUM.
  - ACT copies X^T to SBUF.
  - One fp32 matmul per tile: lhsT = X^T slice, rhs = Mbd, where Mbd is the
    128x128 block-diagonal matrix with 8 copies of Wf.T.  Output lands in
    row-major layout directly:  out[p, g*16+j] = sum_d X[p, g*16+d] Wf[j,d].
  - DVE evicts PSUM -> SBUF fused with the bias add (bias tiled 32x per row).
  - DMA out.  Only chem is ever read -> 32MB of HBM traffic per core.
"""

import sys

sys.path.insert(0, "/opt/trn_rl_repo")

import numpy as np

import concourse.bacc as bacc
import concourse.mybir as mybir
import concourse.tile as tile
from concourse.bass_utils import run_bass_kernel_spmd

B = 2097152
DIM = 16
N_CORES = 8
ROWS = B // N_CORES            # 262144 rows per core
FLAT = ROWS * DIM              # 4194304 fp32 per core
CHUNK_FREE = 2048              # per-partition floats per DMA chunk (8KB)
N_CHUNKS = FLAT // (128 * CHUNK_FREE)   # 16 chunks of 1MB
TILES_PER_CHUNK = CHUNK_FREE // 128     # 16
F32 = mybir.dt.float32


def build_nc(n_chunks=N_CHUNKS, chunk_free=CHUNK_FREE):
    rows = n_chunks * 128 * chunk_free // DIM
    nc = bacc.Bacc(
        "TRN2",
        target_bir_lowering=False,
        debug=False,
        enable_asserts=False,
        num_devices=N_CORES,
    )
    x = nc.dram_tensor("x", [rows, DIM], F32, kind="ExternalInput").ap()
    y = nc.dram_tensor("y", [rows, DIM], F32, kind="ExternalOutput").ap()
    mbd = nc.dram_tensor("mbd", [128, 128], F32, kind="ExternalInput").ap()
    bias = nc.dram_tensor("bias", [128, 512], F32, kind="ExternalInput").ap()
    ident = nc.dram_tensor("ident", [128, 128], F32, kind="ExternalInput").ap()

    e = chunk_free // DIM
    xv = x.rearrange("(c p e) d -> c p (e d)", p=128, e=e)
    yv = y.rearrange("(c p e) d -> c p (e d)", p=128, e=e)
    quads = chunk_free // 512

    with tile.TileContext(nc) as tc:
        with (
            tc.tile_pool(name="consts", bufs=1) as consts,
            tc.tile_pool(name="xin", bufs=3) as xin_pool,
            tc.tile_pool(name="xt", bufs=4) as xt_pool,
            tc.tile_pool(name="yout", bufs=3) as yout_pool,
            tc.tile_pool(name="ps1", bufs=3, space="PSUM") as ps1_pool,
            tc.tile_pool(name="ps2", bufs=3, space="PSUM") as ps2_pool,
        ):
            mbd_sb = consts.tile([128, 128], F32)
            nc.sync.dma_start(out=mbd_sb[:], in_=mbd)
            bias_sb = consts.tile([128, 512], F32)
            nc.sync.dma_start(out=bias_sb[:], in_=bias)
            id_sb = consts.tile([128, 128], F32)
            nc.sync.dma_start(out=id_sb[:], in_=ident)

            for c in range(n_chunks):
                x_sb = xin_pool.tile([128, chunk_free], F32)
                nc.sync.dma_start(out=x_sb[:], in_=xv[c])
                y_sb = yout_pool.tile([128, chunk_free], F32)
                for q in range(quads):
                    ps1 = ps1_pool.tile([128, 512], F32)
                    for t in range(4):
                        col = (q * 4 + t) * 128
                        nc.tensor.transpose(
                            ps1[:, t * 128 : (t + 1) * 128],
                            x_sb[:, col : col + 128],
                            id_sb[:],
                        )
                    xt_sb = xt_pool.tile([128, 512], F32)
                    nc.scalar.copy(out=xt_sb[:], in_=ps1[:])
                    ps2 = ps2_pool.tile([128, 512], F32)
                    for t in range(4):
                        nc.tensor.matmul(
                            ps2[:, t * 128 : (t + 1) * 128],
                            lhsT=xt_sb[:, t * 128 : (t + 1) * 128],
                            rhs=mbd_sb[:],
                            start=True,
                            stop=True,
                        )
                    nc.vector.tensor_add(
                        out=y_sb[:, q * 512 : (q + 1) * 512],
                        in0=ps2[:],
                        in1=bias_sb[:],
                    )
                nc.sync.dma_start(out=yv[c], in_=y_sb[:])
    nc.compile()
    return nc


_NC_CACHE = {}


def _get_nc():
    if "nc" not in _NC_CACHE:
        _NC_CACHE["nc"] = build_nc()
    return _NC_CACHE["nc"]


def make_consts(in_proj_weight, in_proj_bias, out_proj_weight, out_proj_bias):
    Wv = np.asarray(in_proj_weight)[2 * DIM : 3 * DIM].astype(np.float64)
    bv = np.asarray(in_proj_bias)[2 * DIM : 3 * DIM].astype(np.float64)
    Wo = np.asarray(out_proj_weight).astype(np.float64)
    bo = np.asarray(out_proj_bias).astype(np.float64)
    Wf = Wo @ Wv                       # y = x @ Wf.T + bf
    bf = Wo @ bv + bo
    WfT = Wf.T.astype(np.float32)      # [d, j]
    Mbd = np.zeros((128, 128), np.float32)
    for g in range(8):
        Mbd[g * 16 : (g + 1) * 16, g * 16 : (g + 1) * 16] = WfT
    bias_tile = np.broadcast_to(
        np.tile(bf.astype(np.float32), 32), (128, 512)
    ).copy()
    ident = np.eye(128, dtype=np.float32)
    return Mbd, bias_tile, ident


def run(chem, consts, trace=False, **trace_kwargs):
    Mbd, bias_tile, ident = consts
    chem = np.ascontiguousarray(np.asarray(chem), dtype=np.float32)
    assert chem.shape == (B, DIM)
    shards = chem.reshape(N_CORES, ROWS, DIM)
    in_maps = [
        {"x": shards[i], "mbd": Mbd, "bias": bias_tile, "ident": ident}
        for i in range(N_CORES)
    ]
    nc = _get_nc()
    res = run_bass_kernel_spmd(
        nc, in_maps, list(range(N_CORES)), trace=trace, **trace_kwargs
    )
    out = np.concatenate([res.results[i]["y"] for i in range(N_CORES)], axis=0)
    return out, res


def kernel(fp_16, chem_16, in_proj_weight, in_proj_bias, out_proj_weight,
           out_proj_bias):
    consts = make_consts(in_proj_weight, in_proj_bias, out_proj_weight,
                         out_proj_bias)
    out, _ = run(chem_16, consts, trace=False)
    return out


# revision 8
# speedup vs baseline: 1.1496x; 1.1496x over previous
"""Trainium2 Bass kernel for nn_CrossAttention_86165633892747.

Math: seq_len_q = seq_len_kv = 1, so softmax over the length-1 key axis is
exactly 1.0 and attn_out == v.  The whole module collapses to

    out = (chem_16 @ Wv.T + bv) @ Wout.T + bout
        = chem_16 @ (Wout @ Wv).T + (Wout @ bv + bout)

i.e. a single per-row 16x16 linear map.  fp_16 / Wq / Wk / bq / bk are dead.

Device strategy (pure data parallel over 8 cores, B/8 = 262144 rows each):
  - View the row-major (R,16) shard as flat 128x128 fp32 tiles where each
    SBUF partition holds 128 consecutive floats = 8 consecutive rows.
  - PE transpose the tile (fp32 exact, identity matmul) -> X^T in PSUM.
  - ACT copies X^T to SBUF.
  - One fp32 matmul per tile: lhsT = X^T slice, rhs = Mbd, where Mbd is the
    128x128 block-diagonal matrix with 8 copies of Wf.T.  Output lands in
    row-major layout directly:  out[p, g*16+j] = sum_d X[p, g*16+d] Wf[j,d].
  - DVE evicts PSUM -> SBUF fused with the bias add (bias tiled 32x per row).
  - DMA out.  Only chem is ever read -> 32MB of HBM traffic per core.
"""

import sys

sys.path.insert(0, "/opt/trn_rl_repo")

import numpy as np

import concourse.bacc as bacc
import concourse.mybir as mybir
import concourse.tile as tile
from concourse.bass_utils import run_bass_kernel_spmd

B = 2097152
DIM = 16
N_CORES = 8
ROWS = B // N_CORES            # 262144 rows per core
FLAT = ROWS * DIM              # 4194304 fp32 per core
CHUNK_FREE = 2048              # per-partition floats per DMA chunk (8KB)
N_CHUNKS = FLAT // (128 * CHUNK_FREE)   # 16 chunks of 1MB
TILES_PER_CHUNK = CHUNK_FREE // 128     # 16
F32 = mybir.dt.float32


def build_nc(n_chunks=N_CHUNKS, chunk_free=CHUNK_FREE, precision="f32r"):
    """precision: "fp32" = exact two-pass PE matmuls (~1e-7 rel err),
    "f32r" = single-pass FP22-truncated reads (~5e-5 rel err, ~35us less
    PE time; the PE is nearly co-critical with DMA at fp32)."""
    rows = n_chunks * 128 * chunk_free // DIM
    nc = bacc.Bacc(
        "TRN2",
        target_bir_lowering=False,
        debug=False,
        enable_asserts=False,
        num_devices=N_CORES,
    )
    # f32r = "fp32 reduced" (PE truncates reads to FP22/e8m13, single pass).
    # Same bit layout as fp32; the BIR verifier requires every operand of an
    # FP32r matmult to be *declared* f32r at its producer, so the x/mbd/ident
    # tensors and intermediate tiles carry the f32r dtype end-to-end.
    xdt = mybir.dt.float32r if precision == "f32r" else F32
    x = nc.dram_tensor("x", [rows, DIM], xdt, kind="ExternalInput").ap()
    y = nc.dram_tensor("y", [rows, DIM], F32, kind="ExternalOutput").ap()
    mbd = nc.dram_tensor("mbd", [128, 128], xdt, kind="ExternalInput").ap()
    bias = nc.dram_tensor("bias", [128, 512], F32, kind="ExternalInput").ap()
    ident = nc.dram_tensor("ident", [128, 128], xdt, kind="ExternalInput").ap()

    e = chunk_free // DIM
    xv = x.rearrange("(c p e) d -> c p (e d)", p=128, e=e)
    yv = y.rearrange("(c p e) d -> c p (e d)", p=128, e=e)
    quads = chunk_free // 512

    with tile.TileContext(nc) as tc:
        with (
            tc.tile_pool(name="consts", bufs=1) as consts,
            tc.tile_pool(name="xin", bufs=4) as xin_pool,
            tc.tile_pool(name="xt", bufs=8) as xt_pool,
            tc.tile_pool(name="yout", bufs=4) as yout_pool,
            tc.tile_pool(name="ps1", bufs=4, space="PSUM") as ps1_pool,
            tc.tile_pool(name="ps2", bufs=4, space="PSUM") as ps2_pool,
        ):
            mbd_sb = consts.tile([128, 128], xdt)
            nc.sync.dma_start(out=mbd_sb[:], in_=mbd)
            bias_sb = consts.tile([128, 512], F32)
            nc.sync.dma_start(out=bias_sb[:], in_=bias)
            id_sb = consts.tile([128, 128], xdt)
            nc.sync.dma_start(out=id_sb[:], in_=ident)

            for c in range(n_chunks):
                x_sb = xin_pool.tile([128, chunk_free], xdt)
                nc.sync.dma_start(out=x_sb[:], in_=xv[c])
                y_sb = yout_pool.tile([128, chunk_free], F32)
                for q in range(quads):
                    ps1 = ps1_pool.tile([128, 512], xdt)
                    for t in range(4):
                        col = (q * 4 + t) * 128
                        nc.tensor.transpose(
                            ps1[:, t * 128 : (t + 1) * 128],
                            x_sb[:, col : col + 128],
                            id_sb[:],
                        )
                    xt_sb = xt_pool.tile([128, 512], xdt)
                    nc.scalar.copy(out=xt_sb[:], in_=ps1[:])
                    ps2 = ps2_pool.tile([128, 512], F32)
                    for t in range(4):
                        nc.tensor.matmul(
                            ps2[:, t * 128 : (t + 1) * 128],
                            lhsT=xt_sb[:, t * 128 : (t + 1) * 128],
                            rhs=mbd_sb[:],
                            start=True,
                            stop=True,
                        )
                    nc.vector.tensor_add(
                        out=y_sb[:, q * 512 : (q + 1) * 512],
                        in0=ps2[:],
                        in1=bias_sb[:],
                    )
                # stores go out on the ACT HWDGE ring so they don't
                # head-of-line block behind loads on the SP ring
                nc.scalar.dma_start(out=yv[c], in_=y_sb[:])
    nc.compile()
    return nc


_NC_CACHE = {}


def _get_nc():
    if "nc" not in _NC_CACHE:
        _NC_CACHE["nc"] = build_nc()
    return _NC_CACHE["nc"]


def make_consts(in_proj_weight, in_proj_bias, out_proj_weight, out_proj_bias):
    Wv = np.asarray(in_proj_weight)[2 * DIM : 3 * DIM].astype(np.float64)
    bv = np.asarray(in_proj_bias)[2 * DIM : 3 * DIM].astype(np.float64)
    Wo = np.asarray(out_proj_weight).astype(np.float64)
    bo = np.asarray(out_proj_bias).astype(np.float64)
    Wf = Wo @ Wv                       # y = x @ Wf.T + bf
    bf = Wo @ bv + bo
    WfT = Wf.T.astype(np.float32)      # [d, j]
    Mbd = np.zeros((128, 128), np.float32)
    for g in range(8):
        Mbd[g * 16 : (g + 1) * 16, g * 16 : (g + 1) * 16] = WfT
    bias_tile = np.broadcast_to(
        np.tile(bf.astype(np.float32), 32), (128, 512)
    ).copy()
    ident = np.eye(128, dtype=np.float32)
    return Mbd, bias_tile, ident


def run(chem, consts, trace=False, **trace_kwargs):
    Mbd, bias_tile, ident = consts
    chem = np.ascontiguousarray(np.asarray(chem), dtype=np.float32)
    assert chem.shape == (B, DIM)
    shards = chem.reshape(N_CORES, ROWS, DIM)
    in_maps = [
        {"x": shards[i], "mbd": Mbd, "bias": bias_tile, "ident": ident}
        for i in range(N_CORES)
    ]
    nc = _get_nc()
    res = run_bass_kernel_spmd(
        nc, in_maps, list(range(N_CORES)), trace=trace, **trace_kwargs
    )
    out = np.concatenate([res.results[i]["y"] for i in range(N_CORES)], axis=0)
    return out, res


def kernel(fp_16, chem_16, in_proj_weight, in_proj_bias, out_proj_weight,
           out_proj_bias):
    consts = make_consts(in_proj_weight, in_proj_bias, out_proj_weight,
                         out_proj_bias)
    out, _ = run(chem_16, consts, trace=False)
    return out
